# revision 19
# baseline (speedup 1.0000x reference)
"""Trainium2 Bass kernel for nn_Attention_81750407512209.

Full attention: out = softmax((x Wq)(x Wk)^T / sqrt(128)) @ (x Wv)
B=8 batches sharded 1:1 onto 8 NeuronCores (data parallel, weights replicated).

v4 design (vs 345us baseline): engine-balanced software pipeline.
  - Host ships x as a bf16 hi/lo split (x ~= x_hi + x_lo) plus bf16 weights
    (wq pre-scaled by 1/sqrt(128)).  Projections accumulate hi+lo in PSUM,
    keeping Q/K precision near-f32 while all matmuls stream bf16 (1 cyc/col).
  - x^T via HBM->SBUF xbar DMA transposes (no PE transposes, no f32r).
  - Scores per q-tile in chunks (1536,1536,512,512).  c0/c1 live in a
    2-buffered 3-bank pool, c2a/c2b SELF-CYCLE in a dedicated 1-bank pool:
    no chunk's scores ever wait on another chunk family's exp, so every
    PSUM-slot reuse edge is a short intra-family cycle (<5us), not the
    10us whole-chain round trip that plagued earlier versions.
  - Flash-style progressive biases: c0/c1 exp'd with -m01, c2a with -m012,
    c2b with -m0123; AV accumulates in one 129-col PSUM bank (ones column
    of V gives row sums) with two gamma rescales (DVE) spliced between
    kv segments 0..23 / 24..27 / 28..31; gammas exp'd batched [128,2].
  - exp on ScalarE PSUM->SBUF bf16; P^T via one 1MB xbar transpose per tile
    (sync/HWDGE); reciprocal on DVE; normalize on ScalarE; out DMA on sync.
  - 4-stage pipeline: tile i scores/exps, tile i-1 final exp + transpose +
    gamma, tile i-2 AV + rescales + recip, tile i-3 normalize + store.
"""

import numpy as np
import ml_dtypes
from contextlib import ExitStack

import concourse.bass as bass
import concourse.tile as tile
from concourse import bacc, mybir
from concourse.bass_utils import run_bass_kernel_spmd

F32 = mybir.dt.float32
BF16 = mybir.dt.bfloat16
AX = mybir.AxisListType.X
OP = mybir.AluOpType
AF = mybir.ActivationFunctionType

B, N, D = 8, 4096, 128
NT = N // 128                     # 32 kv/q tiles
SCALE = 1.0 / np.sqrt(np.float32(D))
# score chunks (cols): c0,c1 in the 2x3-bank AB pool; c2a,c2b self-cycle in a
# 1-bank pool so no other chunk's scores ever wait on the bias-complete exp
CH = [(0, 1536), (1536, 3072), (3072, 3584), (3584, 4096)]
TS0 = 12                          # AV seg 0: kv 0..11 (c0 @ -m0)
TS1 = 24                          # seg 1: kv 12..23 (c1 @ -m01)
TS2 = 28                          # seg 2: 24..27 (c2a @ -m012); seg 3: 28..31


def build_attention(nc: bacc.Bacc):
    x_hi = nc.dram_tensor("x_hi", [N, D], BF16, kind="ExternalInput").ap()
    x_lo = nc.dram_tensor("x_lo", [N, D], BF16, kind="ExternalInput").ap()
    wqs = nc.dram_tensor("wqs", [D, D], BF16, kind="ExternalInput").ap()
    wkb = nc.dram_tensor("wkb", [D, D], BF16, kind="ExternalInput").ap()
    wvb = nc.dram_tensor("wvb", [D, D], BF16, kind="ExternalInput").ap()
    out = nc.dram_tensor("out", [N, D], F32, kind="ExternalOutput").ap()

    with tile.TileContext(nc) as tc, ExitStack() as ctx:
        consts = ctx.enter_context(tc.tile_pool(name="consts", bufs=1))
        big = ctx.enter_context(tc.tile_pool(name="big", bufs=1))
        pbuf = ctx.enter_context(tc.tile_pool(name="pbuf", bufs=3))
        ptbuf = ctx.enter_context(tc.tile_pool(name="ptbuf", bufs=4))
        stats = ctx.enter_context(tc.tile_pool(name="stats", bufs=6))
        ostage = ctx.enter_context(tc.tile_pool(name="ostage", bufs=4))

        # ---- engine warmup: ACT exp table load, off critical path ----
        dumm = consts.tile([128, 1], F32, name="dumm")
        nc.gpsimd.memset(dumm[:], 0.0)
        dumm2 = consts.tile([128, 1], F32, name="dumm2")
        nc.scalar.activation(dumm2[:], dumm[:], AF.Exp)
        nc.vector.tensor_tensor(dumm2[:], dumm[:], dumm2[:], op=OP.min)

        wq_st = consts.tile([128, 128], BF16, name="wq_st")
        wk_st = consts.tile([128, 128], BF16, name="wk_st")
        wv_st = consts.tile([128, 128], BF16, name="wv_st")
        nc.sync.dma_start(wq_st[:], wqs[:])
        nc.sync.dma_start(wk_st[:], wkb[:])
        nc.sync.dma_start(wv_st[:], wvb[:])

        xT_hi = big.tile([128, N], BF16, name="xT_hi")
        xT_lo = big.tile([128, N], BF16, name="xT_lo")
        kT = big.tile([128, N], BF16, name="kT")
        qT = big.tile([128, N], BF16, name="qT")
        vaug = big.tile([128, NT, 129], BF16, name="vaug")
        nc.gpsimd.memset(vaug[:, :, 128:129], 1.0)

        # ---- prologue: x^T via xbar (HBM->SBUF), projections hi+lo ----
        with tc.tile_pool(name="ps_pro", bufs=3, space="PSUM") as ps_pro:
            for c in range(4):                      # 1024-token quarters
                tsl = slice(c * 1024, (c + 1) * 1024)
                nc.sync.dma_start_transpose(
                    xT_hi[:, tsl], x_hi[tsl, :]
                )
                nc.sync.dma_start_transpose(
                    xT_lo[:, tsl], x_lo[tsl, :]
                )
            for c in range(N // 512):               # 512-token chunks
                sl = slice(c * 512, (c + 1) * 512)
                pk = ps_pro.tile([128, 512], F32, tag="proj", name="pk")
                nc.tensor.matmul(pk[:], wk_st[:], xT_hi[:, sl], start=True, stop=False)
                nc.tensor.matmul(pk[:], wk_st[:], xT_lo[:, sl], start=False, stop=True)
                nc.vector.tensor_copy(kT[:, sl], pk[:])
                pq = ps_pro.tile([128, 512], F32, tag="proj", name="pq")
                nc.tensor.matmul(pq[:], wq_st[:], xT_hi[:, sl], start=True, stop=False)
                nc.tensor.matmul(pq[:], wq_st[:], xT_lo[:, sl], start=False, stop=True)
                nc.scalar.copy(qT[:, sl], pq[:])
                pv = ps_pro.tile([128, 4, 128], F32, tag="vproj", name="pv")
                for u in range(4):
                    t = c * 4 + u
                    dsl = slice(t * 128, (t + 1) * 128)
                    nc.tensor.matmul(
                        pv[:, u, :], xT_hi[:, dsl], wv_st[:], start=True, stop=False
                    )
                    nc.tensor.matmul(
                        pv[:, u, :], xT_lo[:, dsl], wv_st[:], start=False, stop=True
                    )
                nc.vector.tensor_copy(vaug[:, c * 4:(c + 1) * 4, 0:128], pv[:])

        # ---- main loop pools ----
        ps_s = ctx.enter_context(tc.tile_pool(name="ps_s", bufs=2, space="PSUM"))
        ps_c = ctx.enter_context(tc.tile_pool(name="ps_c", bufs=1, space="PSUM"))
        ps_av = ctx.enter_context(tc.tile_pool(name="ps_av", bufs=1, space="PSUM"))

        # per-tile pipeline state, keyed by tile index
        st = {}

        def scores(qsl, lo, hi, s):
            for k in range((hi - lo) // 512):
                nc.tensor.matmul(
                    s[:, k * 512:(k + 1) * 512],
                    qsl,
                    kT[:, lo + k * 512: lo + (k + 1) * 512],
                    start=True,
                    stop=True,
                )

        def negmax(dst_tag, src, width):
            t = stats.tile([128, 1], F32, tag=dst_tag, name=dst_tag)
            nc.vector.reduce_max(t[:], src[:, 0:width], axis=AX, negate=True)
            return t

        def tmin(dst_tag, a, b):
            t = stats.tile([128, 1], F32, tag=dst_tag, name=dst_tag)
            nc.vector.tensor_tensor(t[:], a[:], b[:], op=OP.min)
            return t

        for i in range(NT + 3):
            # --- tail of tile i-3: normalize + store, frees its av bank ---
            if i - 3 >= 0:
                s3 = st[i - 3]
                ost = ostage.tile([128, 128], F32, tag="ost", name="ost")
                nc.scalar.activation(
                    ost[:], s3["av"][:, 0:128], AF.Copy, bias=0.0,
                    scale=s3["linv"][:],
                )
                j = i - 3
                nc.sync.dma_start(out[j * 128:(j + 1) * 128, :], ost[:])
                del st[j]

            # --- tile i-1: last exp (bias n0123) completes P, then transpose ---
            if 0 <= i - 1 < NT:
                s1 = st[i - 1]
                nc.scalar.activation(
                    s1["P"][:, CH[3][0]:CH[3][1]], s1["s2b"][:, 0:512],
                    AF.Exp, bias=s1["n0123"][:],
                )
                PT = ptbuf.tile([128, NT, 128], BF16, tag="PT", name="PT")
                nc.sync.dma_start_transpose(PT[:, 0:NT, :], s1["P"][:, 0:N])
                s1["PT"] = PT

            # --- tile i: scores c0/c1, maxes, exps (bias n01) ---
            if i < NT:
                cur = {}
                st[i] = cur
                qsl = qT[:, i * 128:(i + 1) * 128]
                cur["P"] = pbuf.tile([128, N], BF16, tag="P", name="P")
                with tc.high_priority(offset=40):
                    s0 = ps_s.tile([128, 1536], F32, tag="sh", name="s0")
                    scores(qsl, CH[0][0], CH[0][1], s0)
                    n0 = negmax("n0", s0, 1536)
                    s1c = ps_s.tile([128, 1536], F32, tag="sh", name="s1")
                    scores(qsl, CH[1][0], CH[1][1], s1c)
                    n1 = negmax("n1", s1c, 1536)
                nc.scalar.activation(
                    cur["P"][:, CH[0][0]:CH[0][1]], s0[:, 0:1536],
                    AF.Exp, bias=n0[:],
                )
                n01 = tmin("n01", n0, n1)
                cur["n01"] = n01
                gins = stats.tile([128, 3], F32, tag="gins", name="gins")
                cur["gins"] = gins
                nc.vector.tensor_tensor(
                    gins[:, 0:1], n01[:], n0[:], op=OP.subtract
                )
                nc.scalar.activation(
                    cur["P"][:, CH[1][0]:CH[1][1]], s1c[:, 0:1536],
                    AF.Exp, bias=n01[:],
                )

            # --- tile i-2: AV segment 1 (kv 0..23) ---
            if 0 <= i - 2 < NT:
                s2t = st[i - 2]
                av = ps_av.tile([128, 129], F32, tag="av", name="av")
                s2t["av"] = av
                for t in range(TS0):
                    nc.tensor.matmul(
                        av[:], s2t["PT"][:, t, :], vaug[:, t, :],
                        start=(t == 0), stop=False,
                    )
                nc.scalar.activation(
                    av[:], av[:], AF.Copy, bias=0.0, scale=s2t["gams"][:, 0:1]
                )
                for t in range(TS0, TS1):
                    nc.tensor.matmul(
                        av[:], s2t["PT"][:, t, :], vaug[:, t, :],
                        start=False, stop=False,
                    )

            # --- tile i: scores c2a, stats, exp (bias n012) ---
            if i < NT:
                with tc.high_priority(offset=40):
                    s2a = ps_c.tile([128, 512], F32, tag="sc", name="s2a")
                    scores(qsl, CH[2][0], CH[2][1], s2a)
                    n2a = negmax("n2a", s2a, 512)
                n012 = tmin("n012", n01, n2a)
                cur["n012"] = n012
                nc.vector.tensor_tensor(
                    gins[:, 1:2], n012[:], n01[:], op=OP.subtract
                )
                nc.scalar.activation(
                    cur["P"][:, CH[2][0]:CH[2][1]], s2a[:, 0:512],
                    AF.Exp, bias=n012[:],
                )

            # --- tile i-2: rescale by gamma_1 (DVE), AV seg 2 (kv 24..27) ---
            if 0 <= i - 2 < NT:
                nc.vector.tensor_scalar_mul(
                    s2t["av"][:], s2t["av"][:], s2t["gams"][:, 1:2]
                )
                for t in range(TS1, TS2):
                    nc.tensor.matmul(
                        av[:], s2t["PT"][:, t, :], vaug[:, t, :],
                        start=False, stop=False,
                    )

            # --- tile i: scores c2b, stats ---
            if i < NT:
                with tc.high_priority(offset=40):
                    s2b = ps_c.tile([128, 512], F32, tag="sc", name="s2b")
                    scores(qsl, CH[3][0], CH[3][1], s2b)
                    cur["s2b"] = s2b
                    n2b = negmax("n2b", s2b, 512)
                n0123 = tmin("n0123", n012, n2b)
                cur["n0123"] = n0123
                nc.vector.tensor_tensor(
                    gins[:, 2:3], n0123[:], n012[:], op=OP.subtract
                )

            # --- tile i-2: rescale by gamma_2 (ScalarE), AV seg 3 (kv 28..31) ---
            if 0 <= i - 2 < NT:
                nc.scalar.activation(
                    s2t["av"][:], s2t["av"][:], AF.Copy, bias=0.0,
                    scale=s2t["gams"][:, 2:3],
                )
                for t in range(TS2, NT):
                    nc.tensor.matmul(
                        av[:], s2t["PT"][:, t, :], vaug[:, t, :],
                        start=False, stop=(t == NT - 1),
                    )
                linv = stats.tile([128, 1], F32, tag="linv", name="linv")
                nc.vector.reciprocal(linv[:], av[:, 128:129])
                s2t["linv"] = linv

            # --- tile i-1: batched gamma exps ---
            if 0 <= i - 1 < NT:
                gams = stats.tile([128, 3], F32, tag="gams", name="gams")
                nc.scalar.activation(gams[:], s1["gins"][:], AF.Exp)
                s1["gams"] = gams

    nc.compile()
    return nc


_NC_CACHE = {}


def _get_nc():
    if "nc" not in _NC_CACHE:
        nc = bacc.Bacc("TRN2", target_bir_lowering=False, debug=False, num_devices=B)
        _NC_CACHE["nc"] = build_attention(nc)
    return _NC_CACHE["nc"]


def kernel(x, w_query, w_key, w_value, _trace=False):
    bf16 = ml_dtypes.bfloat16
    x = np.ascontiguousarray(np.asarray(x, dtype=np.float32))
    w_query = np.asarray(w_query, dtype=np.float32)
    w_key = np.asarray(w_key, dtype=np.float32)
    w_value = np.asarray(w_value, dtype=np.float32)

    x_hi = x.astype(bf16)
    x_lo = (x - x_hi.astype(np.float32)).astype(bf16)
    wqs = (w_query * np.float32(SCALE)).astype(bf16)
    wkb = w_key.astype(bf16)
    wvb = w_value.astype(bf16)

    nc = _get_nc()
    in_maps = [
        {
            "x_hi": np.ascontiguousarray(x_hi[b]),
            "x_lo": np.ascontiguousarray(x_lo[b]),
            "wqs": wqs,
            "wkb": wkb,
            "wvb": wvb,
        }
        for b in range(B)
    ]
    res = run_bass_kernel_spmd(nc, in_maps, core_ids=list(range(B)), trace=_trace)
    out_full = np.stack([res.results[b]["out"] for b in range(B)])
    if _trace:
        kernel.last_exec_time_ns = res.exec_time_ns
    return out_full
